# revision 32
# baseline (speedup 1.0000x reference)
"""Self-attention (Base_OC / SAGAN-style) module on Trainium2, 8 NeuronCores.

Problem: x[4, 64, 64, 512]; per batch element b (N = 4096 tokens, C = 512):
  f = x@wf+bf [N,64]; g = x@wg+bg [N,64]; hv = x@wh+bh [N,256]
  s = g @ f^T [N,N]; beta = softmax(s); o = beta @ hv [N,256]
  att = gamma*(o@wo+bo) + x; y = relu(BN([att,x] @ wc + bc))

Sharding: 8 cores = batch(4) x query-row-halves(2). Each core receives x[b]
permuted so its own 2048 query rows come first (attention is permutation-
invariant over keys), computes the pipeline for those rows, returns [2048,512].

Precision strategy: the attention-internal path is damped by gamma (~0.04), so
the f/g/hv projections and the NxN score matmul run as fp8(e4m3) DoubleRow
matmuls (2 rows/cycle + 2 contraction blocks per instruction = 4x the fp32r
rate). DoubleRow sums two 128-partition contractions out = A_w^T A_i +
B_w^T B_i; the K=64 score matmul uses the second pair for the e4m3
quantization residual of f (stationary [f_hi|f_lo] against [g|g]), recovering
~11-bit f precision for free. The accuracy-critical pieces stay fp32r (4-byte
storage, relaxed-precision PE mode, 4x fp32 rate): exp, the exp@hv
accumulation (whose ones-column produces the softmax denominator), and the
x-side of the output matmul. Softmax skips the max-subtraction (max |logit|
~ 67, exp stays in fp32 range).

Output-stage algebra, folded host-side (BN scale absorbed into both):
  y = cat@wc = att@wc1 + x@wc2 = x@(wc1+wc2) + (gamma*o)@(wo@wc1) + const
so there is no materialized att/z: the y PSUM accumulates four fp32r x-chunk
matmuls plus one fp8-DR matmul of oT8 (gamma folded into the o-normalize)
against the precomputed wo@wc1. Engine placement keeps ScalarE exp-only; all
bias/cast/relu/normalize work runs on DVE.
"""

import numpy as np
import ml_dtypes

import concourse.bacc as bacc
import concourse.mybir as mybir
import concourse.tile as tile
from concourse.bass_utils import run_bass_kernel_spmd

FP = mybir.dt.float32
RR = mybir.dt.float32r
F8 = mybir.dt.float8e4
AF = mybir.ActivationFunctionType
OP = mybir.AluOpType
DR = mybir.MatmulPerfMode.DoubleRow

E4 = ml_dtypes.float8_e4m3


# View an fp32 AP as float32r for 4x-rate PE matmul (only when N >= 256).
def r32(ap):
    return ap.bitcast(RR)


N_FULL, N_OWN, C, D8, D2 = 4096, 2048, 512, 64, 256
NMT = N_FULL // 128   # 32 key tiles
NCT = C // 128        # 4 channel tiles
NET = D2 // 128       # 2 e tiles
NNB = N_OWN // 512    # 4 query blocks per core
HW2 = 258             # hv width: 256 values | ones col | pad (fp32r needs even)
EPS = 1e-3


def build_program(reps=1):
    nc = bacc.Bacc("TRN2", target_bir_lowering=False, debug=False, num_devices=8)

    xt_d = nc.dram_tensor("xt", [C, N_OWN], RR, kind="ExternalInput").ap()
    x8_d = nc.dram_tensor("x8", [C, N_FULL], F8, kind="ExternalInput").ap()
    wfg_d = nc.dram_tensor("wfg8", [C, 128], F8, kind="ExternalInput").ap()
    bfg_d = nc.dram_tensor("bfg", [128, 1], FP, kind="ExternalInput").ap()
    whx_d = nc.dram_tensor("whx8", [C, HW2], F8, kind="ExternalInput").ap()
    bhbc_d = nc.dram_tensor("bhbc", [128, HW2], FP, kind="ExternalInput").ap()
    wzc_d = nc.dram_tensor("wzc8", [D2, C], F8, kind="ExternalInput").ap()
    wcx_d = nc.dram_tensor("wcx", [C, C], RR, kind="ExternalInput").ap()
    bcbc_d = nc.dram_tensor("bcbc", [128, C], FP, kind="ExternalInput").ap()
    ident_d = nc.dram_tensor("identr", [128, 128], RR, kind="ExternalInput").ap()
    gam_d = nc.dram_tensor("gammar", [128, 1], FP, kind="ExternalInput").ap()
    y_d = nc.dram_tensor("y", [N_OWN, C], FP, kind="ExternalOutput").ap()

    with tile.TileContext(nc) as tc:
        with (
            tc.tile_pool(name="consts", bufs=1) as cpool,
            tc.tile_pool(name="big", bufs=1) as bigp,
            tc.tile_pool(name="stream", bufs=2) as sp,
            tc.tile_pool(name="exps", bufs=4) as exp_pool,
            tc.tile_pool(name="gst", bufs=2) as gsp,
            tc.tile_pool(name="psB_s", bufs=2, space="PSUM") as ps_pool,
            tc.tile_pool(name="psB_u", bufs=1, space="PSUM") as pu,
        ):
            # xT holds only the core's own 2048 query rows (residual + y stage);
            # the key side reads x8.
            xT = bigp.tile([128, NCT * N_OWN], RR)    # 32 KB/part
            x8 = bigp.tile([128, NCT * N_FULL], F8)   # 16 KB/part
            # f8: per-mt [f_hi(128) | f_lo(128)] blocks; the DoubleRow matmul
            # sums f_hi^T g + f_lo^T g, i.e. f at ~11-bit precision for free
            f8 = bigp.tile([64, NMT * 256], F8)
            # g8: per-nb [g_nb(512) | g_nb(512)] blocks (pair for f_hi/f_lo)
            g8 = bigp.tile([64, NNB * 1024], F8)
            hv = bigp.tile([128, NMT * HW2], RR)      # 33 KB
            whx_sb = cpool.tile([128, NCT * HW2], F8)
            wfg_sb = cpool.tile([128, NCT * 128], F8)
            bfg_sb = cpool.tile([128, 1], FP)
            bhbc_sb = cpool.tile([128, HW2], FP)



            def dma_x(half, with_xt):
                for t in range(NCT):
                    nc.sync.dma_start(
                        x8[:, t * N_FULL + half * 512: t * N_FULL + (half + 1) * 512],
                        x8_d[t * 128:(t + 1) * 128, half * 512:(half + 1) * 512])
                if with_xt:
                    for t in range(NCT):
                        nc.sync.dma_start(
                            xT[:, t * N_OWN + half * 512: t * N_OWN + (half + 1) * 512],
                            xt_d[t * 128:(t + 1) * 128, half * 512:(half + 1) * 512])

            # critical-path-first DMA order: wfg/whx + the first x8 block so the
            # fg/hv chains can start almost immediately; xT is deferred (first
            # needed by emit_tail/emit_y).
            nc.sync.dma_start(bfg_sb, bfg_d)
            for ct in range(NCT):
                nc.sync.dma_start(wfg_sb[:, ct * 128:(ct + 1) * 128],
                                  wfg_d[ct * 128:(ct + 1) * 128, :])
                nc.sync.dma_start(
                    x8[:, ct * N_FULL: ct * N_FULL + 512],
                    x8_d[ct * 128:(ct + 1) * 128, 0:512])
                nc.sync.dma_start(whx_sb[:, ct * HW2:(ct + 1) * HW2],
                                  whx_d[ct * 128:(ct + 1) * 128, :])
            nc.sync.dma_start(bhbc_sb, bhbc_d)

            x8v = x8.rearrange("p (t n) -> p t n", t=NCT)

            def x8_mov(j, half):
                # moving [128, 2, 512]: ct pair (2j, 2j+1) for key block `half`
                return x8v[:, 2 * j:2 * j + 2, half * 512:(half + 1) * 512]

            def x8_stat(j, mt):
                # stationary [128, 2, 128]: ct pair (2j, 2j+1) for key tile mt
                return x8v[:, 2 * j:2 * j + 2, mt * 128:(mt + 1) * 128]

            def emit_hv(mt, phv):
                hp = phv.tile([128, HW2], FP, tag="hv")
                for j in range(2):
                    nc.tensor.matmul(
                        hp, x8_stat(j, mt),
                        whx_sb[:, 2 * j * HW2:(2 * j + 2) * HW2]
                        .rearrange("p (t n) -> p t n", t=2),
                        start=(j == 0), stop=(j == 1), perf_mode=DR)
                # bias (+ones col) via broadcast add, casts to fp32r
                nc.vector.tensor_add(hv[:, mt * HW2:(mt + 1) * HW2], hp, bhbc_sb)

            def emit_fg(ch, pfg):
                # packed [f|g] projection: out rows 0:64 = f, 64:128 = g
                fgp = pfg.tile([128, 512], FP, tag="fg")
                for j in range(2):
                    nc.tensor.matmul(
                        fgp,
                        wfg_sb[:, 2 * j * 128:(2 * j + 2) * 128]
                        .rearrange("p (t m) -> p t m", t=2),
                        x8_mov(j, ch),
                        start=(j == 0), stop=(j == 1), perf_mode=DR)
                # f -> f8 interleaved mt blocks [f_hi(128)|f_lo(128)] x4 (DVE:
                # bias add + fp8 cast; ScalarE is reserved for the exp stream)
                fhi = f8[0:D8, 4 * ch * 256: (4 * ch + 4) * 256] \
                    .rearrange("p (m n) -> p m n", m=4)[:, :, 0:128]
                flo = f8[0:D8, 4 * ch * 256: (4 * ch + 4) * 256] \
                    .rearrange("p (m n) -> p m n", m=4)[:, :, 128:256]
                fin = fgp[0:D8, :].rearrange("p (m n) -> p m n", m=4)
                nc.vector.tensor_scalar_add(fhi, fin, bfg_sb[0:D8, :])
                # f_lo = e4m3((f + bf) - f_hi): quantization residual
                nc.vector.scalar_tensor_tensor(
                    flo, fin, bfg_sb[0:D8, :], fhi,
                    op0=OP.add, op1=OP.subtract)
                if ch < NNB:
                    # g lives on PSUM rows 64:128; cast there then DMA-shift
                    # down to g8 rows 0:64 (engines cannot cross partitions).
                    gst = gsp.tile([128, 512], F8, tag="gst")
                    nc.vector.tensor_scalar_add(gst[D8:128, :], fgp[D8:128, :],
                                                bfg_sb[D8:128, :])
                    nc.sync.dma_start(g8[:, ch * 1024: ch * 1024 + 512],
                                      gst[D8:128, :])
                    nc.sync.dma_start(g8[:, ch * 1024 + 512:(ch + 1) * 1024],
                                      gst[D8:128, :])

            def emit_su(nb, mt2, up):
                exs = []
                for half in range(2):
                    mt = 2 * mt2 + half
                    sps = ps_pool.tile([128, 512], FP, tag="s")
                    nc.tensor.matmul(
                        sps,
                        f8[:, mt * 256:(mt + 1) * 256]
                        .rearrange("p (t m) -> p t m", t=2),
                        g8[:, nb * 1024:(nb + 1) * 1024]
                        .rearrange("p (t n) -> p t n", t=2),
                        start=True, stop=True, perf_mode=DR)
                    ex = exp_pool.tile([128, 512], RR, tag="expS")
                    nc.scalar.activation(ex, sps, AF.Exp)
                    exs.append(ex)
                for half in range(2):
                    mt = 2 * mt2 + half
                    for ns in range(4):
                        nc.tensor.matmul(
                            up[:, ns * 512: ns * 512 + HW2],
                            r32(exs[half][:, ns * 128:(ns + 1) * 128]),
                            r32(hv[:, mt * HW2:(mt + 1) * HW2]),
                            start=(mt == 0), stop=(mt == NMT - 1))

            def emit_tail(nb, up, pm):
                # normalize (with gamma folded in) -> oT (PE transpose) -> oT8
                oT = sp.tile([128, NET * 512], FP, tag="oT")
                for ns in range(4):
                    rcp = sp.tile([128, 1], FP, tag="rcp")
                    nc.vector.reciprocal(rcp, up[:, ns * 512 + 256: ns * 512 + 257])
                    ob = sp.tile([128, D2], RR, tag="ob")
                    # ob = gamma * u / denom
                    nc.vector.tensor_scalar(
                        ob, up[:, ns * 512: ns * 512 + 256], rcp, gam_sb,
                        op0=OP.mult, op1=OP.mult)
                    for et in range(NET):
                        tp2f = pm.tile([128, 512], FP, tag="m", name="tp2")
                        tp2 = tp2f[:, 0:128]
                        nc.tensor.transpose(
                            r32(tp2), ob[:, et * 128:(et + 1) * 128], ident)
                        nc.vector.tensor_copy(
                            oT[:, et * 512 + ns * 128: et * 512 + (ns + 1) * 128], tp2)
                oT8 = sp.tile([128, NET * 512], F8, tag="oT8")
                nc.vector.tensor_copy(oT8, oT)
                return oT8

            def emit_y(nb, oT8, pm):
                # y = x@(wc_att+wc_x) + (gamma*o)@(wo@wc_att) + bias, relu'd.
                oTv = oT8.rearrange("p (t n) -> p t n", t=NET)
                wzv = wzc_sb.rearrange("p (t d) -> p t d", t=NET)
                for ns in range(4):
                    yp = pm.tile([128, 512], FP, tag="m")
                    for ct in range(NCT):
                        nc.tensor.matmul(
                            yp,
                            r32(xT[:, ct * N_OWN + nb * 512 + ns * 128:
                                ct * N_OWN + nb * 512 + (ns + 1) * 128]),
                            r32(wcx_sb[:, ct * C:(ct + 1) * C]),
                            start=(ct == 0), stop=False)
                    nc.tensor.matmul(
                        yp, oTv[:, :, ns * 128:(ns + 1) * 128], wzv,
                        start=False, stop=True, perf_mode=DR)
                    yb = sp.tile([128, C], FP, tag="yb")
                    nc.vector.tensor_add(yb, yp, bcbc_sb)
                    ys = sp.tile([128, C], FP, tag="ys")
                    nc.vector.tensor_scalar_max(ys, yb, 0.0)
                    nc.sync.dma_start(
                        y_d[nb * 512 + ns * 128: nb * 512 + (ns + 1) * 128, :], ys)

            for _rep in range(reps):
                # ---- merged projections + first query block's s/exp/u pipeline ----
                with (
                    tc.tile_pool(name="psA_fg", bufs=1, space="PSUM") as pfg,
                    tc.tile_pool(name="psA_hv", bufs=1, space="PSUM") as phv,
                ):
                    up0 = pu.tile([128, 2048], FP, tag="u")
                    for ch in range(8):
                        if ch > 0 or _rep > 0:
                            dma_x(ch, with_xt=(ch < 4 and (ch >= 2 or _rep > 0)))
                        emit_fg(ch, pfg)
                        emit_hv(4 * ch, phv)
                        emit_hv(4 * ch + 1, phv)
                        emit_su(0, 2 * ch, up0)
                        emit_hv(4 * ch + 2, phv)
                        emit_hv(4 * ch + 3, phv)
                        if ch == 1 and _rep == 0:
                            # deferred xT loads for query blocks 0/1 + tail consts
                            for t in range(NCT):
                                nc.sync.dma_start(
                                    xT[:, t * N_OWN: t * N_OWN + 1024],
                                    xt_d[t * 128:(t + 1) * 128, 0:1024])
                            ident = cpool.tile([128, 128], RR)
                            nc.sync.dma_start(ident, ident_d)
                            wzc_sb = cpool.tile([128, NET * C], F8)
                            nc.sync.dma_start(
                                wzc_sb.rearrange("p (t d) -> p t d", t=NET),
                                wzc_d.rearrange("(t p) d -> p t d", p=128))
                            gam_sb = cpool.tile([128, 1], FP)
                            nc.sync.dma_start(gam_sb, gam_d)
                        if ch == 3 and _rep == 0:
                            wcx_sb = cpool.tile([128, NCT * C], RR)
                            nc.sync.dma_start(
                                wcx_sb.rearrange("p (t d) -> p t d", t=NCT),
                                wcx_d.rearrange("(t p) d -> p t d", p=128))
                            bcbc_sb = cpool.tile([128, C], FP)
                            nc.sync.dma_start(bcbc_sb, bcbc_d)
                        emit_su(0, 2 * ch + 1, up0)

                # ---- remaining query blocks, y(nb-1) pipelined into u-loop(nb) ----
                with tc.tile_pool(name="psB_m", bufs=2, space="PSUM") as pm:
                    oT_prev = emit_tail(0, up0, pm)
                    for nb in range(1, NNB):
                        up = pu.tile([128, 2048], FP, tag="u")
                        for mt2 in range(NMT // 2):
                            emit_su(nb, mt2, up)
                        emit_y(nb - 1, oT_prev, pm)
                        oT_prev = emit_tail(nb, up, pm)
                    emit_y(NNB - 1, oT_prev, pm)

    nc.compile()
    return nc


_PROG = None


def _get_prog():
    global _PROG
    if _PROG is None:
        _PROG = build_program()
    return _PROG


def make_in_maps(x, wf, bf, wg, bg, wh, bh, wo, bo, gamma, wc, bc,
                 bn_scale, bn_bias, bn_mean, bn_var):
    f32 = lambda a: np.ascontiguousarray(np.asarray(a, dtype=np.float32))
    f8 = lambda a: np.ascontiguousarray(np.asarray(a, dtype=np.float32).astype(E4))
    x = f32(x)
    B = x.shape[0]
    xf = x.reshape(B, N_FULL, C)
    gv = float(np.asarray(gamma).ravel()[0])
    sp_ = np.asarray(bn_scale, np.float64) / np.sqrt(np.asarray(bn_var, np.float64) + EPS)
    # y = cat@wc = att@wc1 + x@wc2 with att = gamma*(o@wo+bo) + x, so
    #   y = x@(wc1+wc2) + (gamma*o)@(wo@wc1) + gamma*bo@wc1   (BN folded in)
    wc1 = np.asarray(wc, np.float64)[:C] * sp_[None, :]
    wc2 = np.asarray(wc, np.float64)[C:] * sp_[None, :]
    wcx = f32(wc1 + wc2)
    wzc = f8(np.asarray(wo, np.float64) @ wc1)
    gbo_fold = (gv * np.asarray(bo, np.float64)) @ wc1
    bcrow = f32(((np.asarray(bc, np.float64) - np.asarray(bn_mean, np.float64)) * sp_
                 + np.asarray(bn_bias, np.float64) + gbo_fold)[None, :])
    whx = np.concatenate([np.asarray(wh, np.float32),
                          np.zeros((C, 2), np.float32)], axis=1)
    bh_row = np.concatenate([np.asarray(bh, np.float32).ravel(),
                             [1.0, 0.0]]).astype(np.float32)
    common = dict(
        wfg8=f8(np.concatenate([np.asarray(wf, np.float32),
                                np.asarray(wg, np.float32)], axis=1)),
        bfg=f32(np.concatenate([np.asarray(bf, np.float32).ravel(),
                                np.asarray(bg, np.float32).ravel()])).reshape(128, 1),
        whx8=f8(whx),
        bhbc=np.broadcast_to(bh_row, (128, HW2)).copy(),
        wzc8=wzc,
        wcx=wcx, bcbc=np.broadcast_to(bcrow, (128, C)).copy(),
        identr=np.eye(128, dtype=np.float32),
        gammar=np.full((128, 1), gv, np.float32),
    )
    in_maps = []
    for core in range(8):
        b, h = core // 2, core % 2
        own = xf[b, h * N_OWN:(h + 1) * N_OWN]
        oth = xf[b, (1 - h) * N_OWN:(2 - h) * N_OWN]
        xp = np.ascontiguousarray(np.concatenate([own, oth], axis=0).T)
        in_maps.append({"xt": np.ascontiguousarray(xp[:, :N_OWN]),
                        "x8": xp.astype(E4), **common})
    return in_maps, B


def assemble(results, B):
    out = np.empty((B, N_FULL, C), np.float32)
    for core in range(8):
        b, h = core // 2, core % 2
        out[b, h * N_OWN:(h + 1) * N_OWN] = results[core]["y"]
    return out.reshape(B, 64, 64, C)


def kernel(**inputs):
    in_maps, B = make_in_maps(**inputs)
    nc = _get_prog()
    res = run_bass_kernel_spmd(nc, in_maps, core_ids=list(range(8)))
    return assemble(res.results, B)


# revision 37
# speedup vs baseline: 1.0094x; 1.0094x over previous
"""Self-attention (Base_OC / SAGAN-style) module on Trainium2, 8 NeuronCores.

Problem: x[4, 64, 64, 512]; per batch element b (N = 4096 tokens, C = 512):
  f = x@wf+bf [N,64]; g = x@wg+bg [N,64]; hv = x@wh+bh [N,256]
  s = g @ f^T [N,N]; beta = softmax(s); o = beta @ hv [N,256]
  att = gamma*(o@wo+bo) + x; y = relu(BN([att,x] @ wc + bc))

Sharding: 8 cores = batch(4) x query-row-halves(2). Each core receives x[b]
permuted so its own 2048 query rows come first (attention is permutation-
invariant over keys), computes the pipeline for those rows, returns [2048,512].

Precision strategy: the attention-internal path is damped by gamma (~0.04), so
the f/g/hv projections and the NxN score matmul run as fp8(e4m3) DoubleRow
matmuls (2 rows/cycle + 2 contraction blocks per instruction = 4x the fp32r
rate). DoubleRow sums two 128-partition contractions out = A_w^T A_i +
B_w^T B_i; the K=64 score matmul uses the second pair for the e4m3
quantization residual of f (stationary [f_hi|f_lo] against [g|g]), recovering
~11-bit f precision for free. The accuracy-critical pieces stay fp32r (4-byte
storage, relaxed-precision PE mode, 4x fp32 rate): exp, the exp@hv
accumulation (whose ones-column produces the softmax denominator), and the
x-side of the output matmul. Softmax skips the max-subtraction (max |logit|
~ 67, exp stays in fp32 range).

Output-stage algebra, folded host-side (BN scale absorbed into both):
  y = cat@wc = att@wc1 + x@wc2 = x@(wc1+wc2) + (gamma*o)@(wo@wc1) + const
so there is no materialized att/z: the y PSUM accumulates four fp32r x-chunk
matmuls plus one fp8-DR matmul of oT8 (gamma folded into the o-normalize)
against the precomputed wo@wc1. Engine placement keeps ScalarE exp-only; all
bias/cast/relu/normalize work runs on DVE.
"""

import numpy as np
import ml_dtypes

import concourse.bacc as bacc
import concourse.mybir as mybir
import concourse.tile as tile
from concourse.bass_utils import run_bass_kernel_spmd

FP = mybir.dt.float32
RR = mybir.dt.float32r
F8 = mybir.dt.float8e4
AF = mybir.ActivationFunctionType
OP = mybir.AluOpType
DR = mybir.MatmulPerfMode.DoubleRow

E4 = ml_dtypes.float8_e4m3


# View an fp32 AP as float32r for 4x-rate PE matmul (only when N >= 256).
def r32(ap):
    return ap.bitcast(RR)


N_FULL, N_OWN, C, D8, D2 = 4096, 2048, 512, 64, 256
NMT = N_FULL // 128   # 32 key tiles
NCT = C // 128        # 4 channel tiles
NET = D2 // 128       # 2 e tiles
NNB = N_OWN // 512    # 4 query blocks per core
HW2 = 258             # hv width: 256 values | ones col | pad (fp32r needs even)
EPS = 1e-3


def build_program(reps=1):
    nc = bacc.Bacc("TRN2", target_bir_lowering=False, debug=False, num_devices=8)

    xt_d = nc.dram_tensor("xt", [C, N_OWN], RR, kind="ExternalInput").ap()
    x8_d = nc.dram_tensor("x8", [C, N_FULL], F8, kind="ExternalInput").ap()
    wfg_d = nc.dram_tensor("wfg8", [C, 128], F8, kind="ExternalInput").ap()
    bfg_d = nc.dram_tensor("bfg", [128, 1], FP, kind="ExternalInput").ap()
    whx_d = nc.dram_tensor("whx8", [C, HW2], F8, kind="ExternalInput").ap()
    bhbc_d = nc.dram_tensor("bhbc", [128, HW2], FP, kind="ExternalInput").ap()
    wzc_d = nc.dram_tensor("wzc8", [D2, C], F8, kind="ExternalInput").ap()
    wcx_d = nc.dram_tensor("wcx", [C, C], RR, kind="ExternalInput").ap()
    bcbc_d = nc.dram_tensor("bcbc", [128, C], FP, kind="ExternalInput").ap()
    ident_d = nc.dram_tensor("identr", [128, 128], F8, kind="ExternalInput").ap()
    gam_d = nc.dram_tensor("gammar", [128, 1], FP, kind="ExternalInput").ap()
    y_d = nc.dram_tensor("y", [N_OWN, C], FP, kind="ExternalOutput").ap()

    with tile.TileContext(nc) as tc:
        with (
            tc.tile_pool(name="consts", bufs=1) as cpool,
            tc.tile_pool(name="big", bufs=1) as bigp,
            tc.tile_pool(name="stream", bufs=2) as sp,
            tc.tile_pool(name="exps", bufs=4) as exp_pool,
            tc.tile_pool(name="gst", bufs=2) as gsp,
            tc.tile_pool(name="psB_s", bufs=2, space="PSUM") as ps_pool,
            tc.tile_pool(name="psB_u", bufs=1, space="PSUM") as pu,
        ):
            # xT holds only the core's own 2048 query rows (residual + y stage);
            # the key side reads x8.
            xT = bigp.tile([128, NCT * N_OWN], RR)    # 32 KB/part
            x8 = bigp.tile([128, NCT * N_FULL], F8)   # 16 KB/part
            # f8: per-mt [f_hi(128) | f_lo(128)] blocks; the DoubleRow matmul
            # sums f_hi^T g + f_lo^T g, i.e. f at ~11-bit precision for free
            f8 = bigp.tile([64, NMT * 256], F8)
            # g8: per-nb [g_nb(512) | g_nb(512)] blocks (pair for f_hi/f_lo)
            g8 = bigp.tile([64, NNB * 1024], F8)
            hv = bigp.tile([128, NMT * HW2], RR)      # 33 KB
            whx_sb = cpool.tile([128, NCT * HW2], F8)
            wfg_sb = cpool.tile([128, NCT * 128], F8)
            bfg_sb = cpool.tile([128, 1], FP)
            bhbc_sb = cpool.tile([128, HW2], FP)



            def dma_x(half, with_xt):
                for t in range(NCT):
                    nc.sync.dma_start(
                        x8[:, t * N_FULL + half * 512: t * N_FULL + (half + 1) * 512],
                        x8_d[t * 128:(t + 1) * 128, half * 512:(half + 1) * 512])
                if with_xt:
                    for t in range(NCT):
                        nc.sync.dma_start(
                            xT[:, t * N_OWN + half * 512: t * N_OWN + (half + 1) * 512],
                            xt_d[t * 128:(t + 1) * 128, half * 512:(half + 1) * 512])

            # critical-path-first DMA order: wfg/whx + the first x8 block so the
            # fg/hv chains can start almost immediately; xT is deferred (first
            # needed by emit_tail/emit_y).
            nc.sync.dma_start(bfg_sb, bfg_d)
            for ct in range(NCT):
                nc.sync.dma_start(wfg_sb[:, ct * 128:(ct + 1) * 128],
                                  wfg_d[ct * 128:(ct + 1) * 128, :])
                nc.sync.dma_start(
                    x8[:, ct * N_FULL: ct * N_FULL + 512],
                    x8_d[ct * 128:(ct + 1) * 128, 0:512])
                nc.sync.dma_start(whx_sb[:, ct * HW2:(ct + 1) * HW2],
                                  whx_d[ct * 128:(ct + 1) * 128, :])
            nc.sync.dma_start(bhbc_sb, bhbc_d)

            x8v = x8.rearrange("p (t n) -> p t n", t=NCT)

            def x8_mov(j, half):
                # moving [128, 2, 512]: ct pair (2j, 2j+1) for key block `half`
                return x8v[:, 2 * j:2 * j + 2, half * 512:(half + 1) * 512]

            def x8_stat(j, mt):
                # stationary [128, 2, 128]: ct pair (2j, 2j+1) for key tile mt
                return x8v[:, 2 * j:2 * j + 2, mt * 128:(mt + 1) * 128]

            def emit_hv(mt, phv):
                hp = phv.tile([128, HW2], FP, tag="hv")
                for j in range(2):
                    nc.tensor.matmul(
                        hp, x8_stat(j, mt),
                        whx_sb[:, 2 * j * HW2:(2 * j + 2) * HW2]
                        .rearrange("p (t n) -> p t n", t=2),
                        start=(j == 0), stop=(j == 1), perf_mode=DR)
                # bias (+ones col) via broadcast add, casts to fp32r
                nc.vector.tensor_add(hv[:, mt * HW2:(mt + 1) * HW2], hp, bhbc_sb)

            def emit_fg(ch, pfg):
                # packed [f|g] projection: out rows 0:64 = f, 64:128 = g
                fgp = pfg.tile([128, 512], FP, tag="fg")
                for j in range(2):
                    nc.tensor.matmul(
                        fgp,
                        wfg_sb[:, 2 * j * 128:(2 * j + 2) * 128]
                        .rearrange("p (t m) -> p t m", t=2),
                        x8_mov(j, ch),
                        start=(j == 0), stop=(j == 1), perf_mode=DR)
                # f -> f8 interleaved mt blocks [f_hi(128)|f_lo(128)] x4 (DVE:
                # bias add + fp8 cast; ScalarE is reserved for the exp stream)
                fhi = f8[0:D8, 4 * ch * 256: (4 * ch + 4) * 256] \
                    .rearrange("p (m n) -> p m n", m=4)[:, :, 0:128]
                flo = f8[0:D8, 4 * ch * 256: (4 * ch + 4) * 256] \
                    .rearrange("p (m n) -> p m n", m=4)[:, :, 128:256]
                fin = fgp[0:D8, :].rearrange("p (m n) -> p m n", m=4)
                nc.vector.tensor_scalar_add(fhi, fin, bfg_sb[0:D8, :])
                # f_lo = e4m3((f + bf) - f_hi): quantization residual
                nc.vector.scalar_tensor_tensor(
                    flo, fin, bfg_sb[0:D8, :], fhi,
                    op0=OP.add, op1=OP.subtract)
                if ch < NNB:
                    # g lives on PSUM rows 64:128; cast there then DMA-shift
                    # down to g8 rows 0:64 (engines cannot cross partitions).
                    gst = gsp.tile([128, 512], F8, tag="gst")
                    nc.vector.tensor_scalar_add(gst[D8:128, :], fgp[D8:128, :],
                                                bfg_sb[D8:128, :])
                    nc.sync.dma_start(g8[:, ch * 1024: ch * 1024 + 512],
                                      gst[D8:128, :])
                    nc.sync.dma_start(g8[:, ch * 1024 + 512:(ch + 1) * 1024],
                                      gst[D8:128, :])

            def emit_su(nb, mt2, up):
                exs = []
                for half in range(2):
                    mt = 2 * mt2 + half
                    sps = ps_pool.tile([128, 512], FP, tag="s")
                    nc.tensor.matmul(
                        sps,
                        f8[:, mt * 256:(mt + 1) * 256]
                        .rearrange("p (t m) -> p t m", t=2),
                        g8[:, nb * 1024:(nb + 1) * 1024]
                        .rearrange("p (t n) -> p t n", t=2),
                        start=True, stop=True, perf_mode=DR)
                    ex = exp_pool.tile([128, 512], RR, tag="expS")
                    nc.scalar.activation(ex, sps, AF.Exp)
                    exs.append(ex)
                for half in range(2):
                    mt = 2 * mt2 + half
                    for ns in range(4):
                        nc.tensor.matmul(
                            up[:, ns * 512: ns * 512 + HW2],
                            r32(exs[half][:, ns * 128:(ns + 1) * 128]),
                            r32(hv[:, mt * HW2:(mt + 1) * HW2]),
                            start=(mt == 0), stop=(mt == NMT - 1))

            def emit_tail(nb, up, pm):
                # normalize (with gamma folded in) -> fp8 -> oT8 (fp8 PE transpose)
                oT8 = sp.tile([128, NET * 512], F8, tag="oT8")
                for ns in range(4):
                    rcp = sp.tile([128, 1], FP, tag="rcp")
                    nc.vector.reciprocal(rcp, up[:, ns * 512 + 256: ns * 512 + 257])
                    ob8 = sp.tile([128, D2], F8, tag="ob")
                    # ob = gamma * u / denom, cast to e4m3
                    nc.vector.tensor_scalar(
                        ob8, up[:, ns * 512: ns * 512 + 256], rcp, gam_sb,
                        op0=OP.mult, op1=OP.mult)
                    for et in range(NET):
                        tp2f = pm.tile([128, 512], FP, tag="m", name="tp2")
                        # fp8 transpose writes with element step 2 (HW packing)
                        tp28 = tp2f.bitcast(F8)[:, 0:256] \
                            .rearrange("p (n two) -> p n two", two=2)[:, :, 0]
                        nc.tensor.transpose(
                            tp28, ob8[:, et * 128:(et + 1) * 128], ident8)
                        nc.vector.tensor_copy(
                            oT8[:, et * 512 + ns * 128: et * 512 + (ns + 1) * 128],
                            tp28)
                return oT8

            def emit_y(nb, oT8, pm):
                # y = x@(wc_att+wc_x) + (gamma*o)@(wo@wc_att) + bias, relu'd.
                oTv = oT8.rearrange("p (t n) -> p t n", t=NET)
                wzv = wzc_sb.rearrange("p (t d) -> p t d", t=NET)
                for ns in range(4):
                    yp = pm.tile([128, 512], FP, tag="m")
                    for ct in range(NCT):
                        nc.tensor.matmul(
                            yp,
                            r32(xT[:, ct * N_OWN + nb * 512 + ns * 128:
                                ct * N_OWN + nb * 512 + (ns + 1) * 128]),
                            r32(wcx_sb[:, ct * C:(ct + 1) * C]),
                            start=(ct == 0), stop=False)
                    nc.tensor.matmul(
                        yp, oTv[:, :, ns * 128:(ns + 1) * 128], wzv,
                        start=False, stop=True, perf_mode=DR)
                    yb = sp.tile([128, C], FP, tag="yb")
                    nc.vector.tensor_add(yb, yp, bcbc_sb)
                    ys = sp.tile([128, C], FP, tag="ys")
                    nc.vector.tensor_scalar_max(ys, yb, 0.0)
                    nc.sync.dma_start(
                        y_d[nb * 512 + ns * 128: nb * 512 + (ns + 1) * 128, :], ys)

            for _rep in range(reps):
                # ---- merged projections + first query block's s/exp/u pipeline ----
                with (
                    tc.tile_pool(name="psA_fg", bufs=1, space="PSUM") as pfg,
                    tc.tile_pool(name="psA_hv", bufs=1, space="PSUM") as phv,
                ):
                    up0 = pu.tile([128, 2048], FP, tag="u")
                    for ch in range(8):
                        if ch > 0 or _rep > 0:
                            dma_x(ch, with_xt=(ch < 4 and (ch >= 2 or _rep > 0)))
                        emit_fg(ch, pfg)
                        emit_hv(4 * ch, phv)
                        emit_hv(4 * ch + 1, phv)
                        emit_su(0, 2 * ch, up0)
                        emit_hv(4 * ch + 2, phv)
                        emit_hv(4 * ch + 3, phv)
                        if ch == 1 and _rep == 0:
                            # deferred xT loads for query blocks 0/1 + tail consts
                            for t in range(NCT):
                                nc.sync.dma_start(
                                    xT[:, t * N_OWN: t * N_OWN + 1024],
                                    xt_d[t * 128:(t + 1) * 128, 0:1024])
                            ident8 = cpool.tile([128, 128], F8)
                            nc.sync.dma_start(ident8, ident_d)
                            wzc_sb = cpool.tile([128, NET * C], F8)
                            nc.sync.dma_start(
                                wzc_sb.rearrange("p (t d) -> p t d", t=NET),
                                wzc_d.rearrange("(t p) d -> p t d", p=128))
                            gam_sb = cpool.tile([128, 1], FP)
                            nc.sync.dma_start(gam_sb, gam_d)
                        if ch == 3 and _rep == 0:
                            wcx_sb = cpool.tile([128, NCT * C], RR)
                            nc.sync.dma_start(
                                wcx_sb.rearrange("p (t d) -> p t d", t=NCT),
                                wcx_d.rearrange("(t p) d -> p t d", p=128))
                            bcbc_sb = cpool.tile([128, C], FP)
                            nc.sync.dma_start(bcbc_sb, bcbc_d)
                        emit_su(0, 2 * ch + 1, up0)

                # ---- remaining query blocks, y(nb-1) pipelined into u-loop(nb) ----
                with tc.tile_pool(name="psB_m", bufs=2, space="PSUM") as pm:
                    oT_prev = emit_tail(0, up0, pm)
                    for nb in range(1, NNB):
                        up = pu.tile([128, 2048], FP, tag="u")
                        for mt2 in range(NMT // 2):
                            emit_su(nb, mt2, up)
                        emit_y(nb - 1, oT_prev, pm)
                        oT_prev = emit_tail(nb, up, pm)
                    emit_y(NNB - 1, oT_prev, pm)

    nc.compile()
    return nc


_PROG = None


def _get_prog():
    global _PROG
    if _PROG is None:
        _PROG = build_program()
    return _PROG


def make_in_maps(x, wf, bf, wg, bg, wh, bh, wo, bo, gamma, wc, bc,
                 bn_scale, bn_bias, bn_mean, bn_var):
    f32 = lambda a: np.ascontiguousarray(np.asarray(a, dtype=np.float32))
    f8 = lambda a: np.ascontiguousarray(np.asarray(a, dtype=np.float32).astype(E4))
    x = f32(x)
    B = x.shape[0]
    xf = x.reshape(B, N_FULL, C)
    gv = float(np.asarray(gamma).ravel()[0])
    sp_ = np.asarray(bn_scale, np.float64) / np.sqrt(np.asarray(bn_var, np.float64) + EPS)
    # y = cat@wc = att@wc1 + x@wc2 with att = gamma*(o@wo+bo) + x, so
    #   y = x@(wc1+wc2) + (gamma*o)@(wo@wc1) + gamma*bo@wc1   (BN folded in)
    wc1 = np.asarray(wc, np.float64)[:C] * sp_[None, :]
    wc2 = np.asarray(wc, np.float64)[C:] * sp_[None, :]
    wcx = f32(wc1 + wc2)
    wzc = f8(np.asarray(wo, np.float64) @ wc1)
    gbo_fold = (gv * np.asarray(bo, np.float64)) @ wc1
    bcrow = f32(((np.asarray(bc, np.float64) - np.asarray(bn_mean, np.float64)) * sp_
                 + np.asarray(bn_bias, np.float64) + gbo_fold)[None, :])
    whx = np.concatenate([np.asarray(wh, np.float32),
                          np.zeros((C, 2), np.float32)], axis=1)
    bh_row = np.concatenate([np.asarray(bh, np.float32).ravel(),
                             [1.0, 0.0]]).astype(np.float32)
    common = dict(
        wfg8=f8(np.concatenate([np.asarray(wf, np.float32),
                                np.asarray(wg, np.float32)], axis=1)),
        bfg=f32(np.concatenate([np.asarray(bf, np.float32).ravel(),
                                np.asarray(bg, np.float32).ravel()])).reshape(128, 1),
        whx8=f8(whx),
        bhbc=np.broadcast_to(bh_row, (128, HW2)).copy(),
        wzc8=wzc,
        wcx=wcx, bcbc=np.broadcast_to(bcrow, (128, C)).copy(),
        identr=np.eye(128, dtype=np.float32).astype(E4),
        gammar=np.full((128, 1), gv, np.float32),
    )
    in_maps = []
    for core in range(8):
        b, h = core // 2, core % 2
        own = xf[b, h * N_OWN:(h + 1) * N_OWN]
        oth = xf[b, (1 - h) * N_OWN:(2 - h) * N_OWN]
        xp = np.ascontiguousarray(np.concatenate([own, oth], axis=0).T)
        in_maps.append({"xt": np.ascontiguousarray(xp[:, :N_OWN]),
                        "x8": xp.astype(E4), **common})
    return in_maps, B


def assemble(results, B):
    out = np.empty((B, N_FULL, C), np.float32)
    for core in range(8):
        b, h = core // 2, core % 2
        out[b, h * N_OWN:(h + 1) * N_OWN] = results[core]["y"]
    return out.reshape(B, 64, 64, C)


def kernel(**inputs):
    in_maps, B = make_in_maps(**inputs)
    nc = _get_prog()
    res = run_bass_kernel_spmd(nc, in_maps, core_ids=list(range(8)))
    return assemble(res.results, B)
